# revision 45
# baseline (speedup 1.0000x reference)
"""Bass/Trainium2 kernel for the LSTM problem (nn_RNN_27685359190558).

Math (per reference):
  xW = x @ W + b                      [B, T, 4H]
  scan over T=28: z = xW_t + h @ U; i,f,g,o = split(z) (Keras order)
      i,f,o = sigmoid; g = relu
      c' = f*c + i*g;  h' = o * relu(c')
  out = softmax(h_final @ Wd + bd)    [B, 10]

Strategy: pure data parallelism over 8 cores (2048 batch each).
On-chip layout is fully transposed: states hT/cT are [H=128 partitions,
batch free].  Per (timestep, 512-batch chunk) and per gate q:
psum[q] = Ur[:,q].T @ hT (fp16) + W-side x-projection.

The x-projection runs as ONE fp8e4 DoubleRow matmul for the first E_FP8
timesteps: plane0 = e4m3(W).T @ e4m3(x), plane1 = e4m3(W/16).T @
e4m3(16*(x - e4m3(x))) -- the residual plane restores x to ~fp16
accuracy, leaving only W's 3% quantization error.  Early-timestep errors
decay through the forget-gate products: E_FP8=24 measures rel err 3.3e-3
(threshold 2e-2) while halving x-proj PE time for those steps (cost
model: DoubleRow = 0.5 cycles/row at free-size 512).  The last T-E_FP8
steps stay fully fp16.

Elementwise phase: sigmoid on ACT covers [i,f,o] per chunk; the g gate
and all c/h updates run on DVE as scalar_tensor_tensor ops -- with all
operands SBUF-resident 2-byte packed, stt runs in the 4x DVE perf mode
(0.25 cycles/elem) vs tensor_tensor's 2x ceiling.  c = f*c + i*relu(g),
h = o*c (c >= 0 always since c0=0, f,i > 0, relu(g) >= 0).

Other measured design points kept from the previous iteration:
- fp16 everywhere on-chip for the non-fp8 path (fp32r streams ~4x slower).
- x zero-padded K=29 -> 128 (K<128 matmuls measured at half column rate).
- Gate order in psum is [i, f, o, g]; one fused ACT sigmoid covers
  [128, 1536] per chunk, written gate-major across a chunk PAIR so the
  pair-wide DVE ops read contiguous 1024-elem spans.
- Bias b folded via a ones-row appended to x (host side).
- Dense matmuls overlap the last timestep; softmax is one wide exp +
  grouped reduce + reciprocal + broadcast multiply.
"""

import sys

sys.path.insert(0, "/opt/trn_rl_repo")

import numpy as np
from contextlib import ExitStack

import concourse.bass as bass
import concourse.bacc as bacc
import concourse.tile as tile
from concourse import mybir
from concourse.bass_utils import run_bass_kernel_spmd

B, T, F, H = 16384, 28, 28, 128
G = 4 * H  # 512
NCLS = 10
NCORES = 8
BC = B // NCORES  # 2048 batch per core
CH = 512  # batch chunk per matmul (one psum bank)
NCH = BC // CH  # 4
FP = F + 1  # 29: features + ones row (bias)
FPAD = 32  # DMA row count: FP padded to the 32-partition alignment
E_FP8 = 27  # timesteps 0..E_FP8-1 use the fp8 DoubleRow x-projection
RSCALE = 16.0  # residual-plane upscale

FP32 = mybir.dt.float32
FP16 = mybir.dt.float16
FP8 = mybir.dt.float8e4  # e4m3

TRACE = False
TIME_REPS = 0  # >0: run cached-executable wall-clock timing after correctness run
LAST_RESULT = None

DR = mybir.MatmulPerfMode.DoubleRow


def _build_kernel(ctx, tc, xT8, xT16, Wdr0, WdrR, Wt16, Ur, Wd, bd, ones1h, out, skip_bias):
    nc = tc.nc
    Sig = mybir.ActivationFunctionType.Sigmoid
    Exp = mybir.ActivationFunctionType.Exp
    mul_op = mybir.AluOpType.mult
    add_op = mybir.AluOpType.add
    max_op = mybir.AluOpType.max

    weights = ctx.enter_context(tc.tile_pool(name="weights", bufs=1))
    state = ctx.enter_context(tc.tile_pool(name="state", bufs=1))
    xpool = ctx.enter_context(tc.tile_pool(name="xpool", bufs=1))
    spool = ctx.enter_context(tc.tile_pool(name="spool", bufs=8))
    tpool = ctx.enter_context(tc.tile_pool(name="tpool", bufs=8))
    opool = ctx.enter_context(tc.tile_pool(name="opool", bufs=2))

    dma = nc.default_dma_engine

    wdr_sb = weights.tile([H, 2, G], FP8)  # plane0 = W8, plane1 = Wlo (x-residual)
    wt_sb = weights.tile([H, G], FP16)  # fp16 W for the tail timesteps
    ur_sb = weights.tile([H, G], FP16)
    wd_sb = weights.tile([H, NCLS], FP16)
    bd_sb = weights.tile([1, NCLS], FP16)
    ones_sb = weights.tile([1, H], FP16)

    # DMA order = queue order.  The ACT queue head carries the activation
    # table loads (~1.3us each), so the two tensors the FIRST matmul needs
    # (DR weights gate i + x0 chunk 0) go at the heads of the SP and Pool
    # queues instead; everything else follows on SP.
    dma.dma_start(out=wdr_sb[:, :, 0:H], in_=Wdr0[:])
    # x tiles: [H partitions, 2 planes, BC] fp8.  Rows FP..127 stay zero
    # (memset once); DMA rewrites rows 0..FP-1 each use.  The same bytes
    # serve the fp16 tail steps via a bitcast view [H, BC] fp16.
    xbufs = [xpool.tile([H, 2, BC], FP8, name=f"xtbuf{j}") for j in range(4)]
    xv16 = [
        xb[:].rearrange("p a b -> p (a b)").bitcast(FP16) for xb in xbufs
    ]  # [H, BC] fp16 views
    warm_src = weights.tile([H, H], FP16)
    nc.gpsimd.memset(warm_src[:], 0.0)
    # Zero the x tiles on DVE instead of gpsimd: the 3.4us-per-buffer Pool
    # memsets serialized the first x DMA to t=3.7us.  DVE (in0 AND 0.0)
    # writes exact zeros even from uninitialized (possibly NaN) bits in
    # 593ns per buffer -- only the pad rows FP..127 actually need it, but a
    # partition slice starting above 0 is capped at 32 partitions, so zero
    # the full tile; the brief x0-DMA WAR wait is negligible.
    for j in range(4):
        full = xbufs[j][:].rearrange("p a b -> p (a b)").bitcast(FP16)
        nc.vector.tensor_scalar(
            out=full,
            in0=full,
            scalar1=0.0,
            scalar2=None,
            op0=mybir.AluOpType.logical_and,
        )
    xt0 = xbufs[0]
    nc.gpsimd.dma_start(out=xt0[0:FPAD, :, 0:CH], in_=xT8[0][:, :, 0:CH])
    for c in range(1, NCH):
        dma.dma_start(
            out=xt0[0:FPAD, :, c * CH : (c + 1) * CH],
            in_=xT8[0][:, :, c * CH : (c + 1) * CH],
        )
    dma.dma_start(out=wdr_sb[:, :, H:G], in_=WdrR[:])
    # prefetch t=1/t=2's x right behind t=0's; t>=3 prefetches issue inside
    # the t-loop three timesteps ahead on the Pool queue.
    dma.dma_start(out=xbufs[1][0:FPAD, :, :], in_=xT8[1])
    dma.dma_start(out=xbufs[2][0:FPAD, :, :], in_=xT8[2])
    dma.dma_start(out=ur_sb[:], in_=Ur[:])
    dma.dma_start(out=wt_sb[:], in_=Wt16[:])
    dma.dma_start(out=wd_sb[:], in_=Wd[:])
    dma.dma_start(out=bd_sb[:], in_=bd[:])
    dma.dma_start(out=ones_sb[:], in_=ones1h[:])

    hT = state.tile([H, BC], FP16)
    cT = state.tile([H, BC], FP16)

    last_h = [None]  # name of the most recent h-producing tt (see finish_pair)

    def finish_pair(p, sp, t1p):
        # Pair-wide (1024-elem) c = f*c, c += i*relu(g), h = c*o -- on DVE
        # tensor_tensor (2x_1p mode; scalar_tensor_tensor gets NO fast mode,
        # so fusing ops into stt is a net loss).  The h write is on the next
        # timestep's critical path (U-matmuls read it), so later g-stts get
        # a no-sync dep on it to keep the ready-heap from reordering.
        # c=f*c runs on the otherwise-idle Pool engine (SBUF-only operands
        # -- legal on GPSIMD; measured 853ns/1024) IN PARALLEL with the
        # g-stt on DVE, shortening the serial post-sigmoid segment on the
        # recurrence lane and relieving the DVE queue.
        p0, p1 = p * 2 * CH, (p + 1) * 2 * CH
        sp3 = sp[:].rearrange("h (g w) -> h g w", g=3)
        eng = nc.gpsimd
        i1 = eng.tensor_tensor(
            out=cT[:, p0:p1], in0=sp3[:, 1, :], in1=cT[:, p0:p1], op=mul_op
        )
        i2 = nc.vector.tensor_tensor(
            out=cT[:, p0:p1], in0=cT[:, p0:p1], in1=t1p[:], op=add_op
        )
        i3 = nc.vector.tensor_tensor(
            out=hT[:, p0:p1], in0=cT[:, p0:p1], in1=sp3[:, 2, :], op=mul_op
        )
        last_h[0] = i3.ins.name

    def finish_single(c, sp, t1p):
        # tail-only per-chunk finish (shorter drain at t=T-1)
        c0, c1 = c * CH, (c + 1) * CH
        half = c % 2
        sp3 = sp[:].rearrange("h (g w) -> h g w", g=3)
        nc.vector.tensor_tensor(
            out=cT[:, c0:c1],
            in0=cT[:, c0:c1],
            in1=t1p[:, half * CH : (half + 1) * CH],
            op=add_op,
        )
        nc.vector.tensor_tensor(
            out=hT[:, c0:c1],
            in0=cT[:, c0:c1],
            in1=sp3[:, 2, half * CH : (half + 1) * CH],
            op=mul_op,
        )

    def emit_dense(pw, blocks, j0):
        for j in blocks:
            d0 = (j - j0) * NCLS
            nc.tensor.matmul(
                pw[:, d0 : d0 + NCLS],
                hT[:, j * H : (j + 1) * H],
                wd_sb[:],
                start=True,
                stop=skip_bias,
            )
            if not skip_bias:
                # + bd via a rank-1 ones @ bd matmul (keeps bias off the DVE)
                nc.tensor.matmul(
                    pw[:, d0 : d0 + NCLS], ones_sb[:], bd_sb[:], start=False, stop=True
                )

    with (
        tc.tile_pool(name="ppool", bufs=2, space="PSUM") as ppool,
        tc.tile_pool(name="gpool", bufs=2, space="PSUM") as gpool,
    ):
        for t in range(T):
            xt = xbufs[t % 4]
            if t + 3 < T:
                # x prefetch at depth 3 (4 rotating bufs): a full timestep of
                # slack so the SP-queue transfer is never on the boundary
                # critical path (depth 2 landed just-in-time).
                j = (t + 3) % 4
                if t + 3 < E_FP8:
                    dma.dma_start(out=xbufs[j][0:FPAD, :, :], in_=xT8[t + 3])
                else:
                    dma.dma_start(out=xv16[j][0:FPAD, :], in_=xT16[t + 3 - E_FP8])
            pending = None
            for c in range(NCH):
                c0, c1 = c * CH, (c + 1) * CH
                half = c % 2
                if half == 0:
                    # s for a chunk PAIR, gate-major: [i0 i1 | f0 f1 | o0 o1]
                    sp = spool.tile([H, 3 * 2 * CH], FP16)
                    t1p = tpool.tile([H, 2 * CH], FP16)
                sp3 = sp[:].rearrange("h (g w) -> h g w", g=3)
                pt = ppool.tile([H, 3 * CH], FP32)
                pg = gpool.tile([H, CH], FP32)

                if t == 0 and c == 0:
                    # PE warmup during the DMA preamble (see warm_src note):
                    # dummy matmuls into the first pt tile; the real DR
                    # matmuls below open with start=True, resetting psum.
                    for _ in range(12):
                        nc.tensor.matmul(
                            pt[:, 0:H],
                            warm_src[:],
                            warm_src[:],
                            start=True,
                            stop=True,
                        )

                # All x-side matmuls FIRST: they depend only on the
                # (prefetched) x DMA, so at the timestep boundary PE streams
                # the x projection while the previous timestep's h is still
                # in flight on DVE.  U-matmuls (h-gated) close the groups;
                # U(g) goes LAST because sigmoid only reads pt (gates i,f,o)
                # -- the g group (pg, read by the Pool stt) can close after
                # sigmoid already started.
                def xdst(q):
                    return pt[:, q * CH : (q + 1) * CH] if q < 3 else pg[:]

                for q in range(4):
                    if t < E_FP8:
                        nc.tensor.matmul(
                            xdst(q),
                            wdr_sb[:, :, q * H : (q + 1) * H],
                            xt[:, :, c0:c1],
                            start=True,
                            stop=(t == 0),
                            perf_mode=DR,
                        )
                    else:
                        nc.tensor.matmul(
                            xdst(q),
                            wt_sb[:, q * H : (q + 1) * H],
                            xv16[t % 4][:, c0:c1],
                            start=True,
                            stop=(t == 0),
                        )
                if t > 0:
                    for q in range(4):
                        nc.tensor.matmul(
                            xdst(q),
                            ur_sb[:, q * H : (q + 1) * H],
                            hT[:, c0:c1],
                            start=False,
                            stop=True,
                        )
                sig_inst = nc.scalar.activation(
                    out=sp3[:, :, half * CH : (half + 1) * CH],
                    in_=pt[:].rearrange("h (g w) -> h g w", g=3),
                    func=Sig,
                )
                if t == 0:
                    # c0 = 0  =>  c' = i * relu(g) = relu(i*g);  h = o*c
                    nc.vector.scalar_tensor_tensor(
                        out=cT[:, c0:c1],
                        in0=pg[:],
                        scalar=0.0,
                        in1=sp3[:, 0, half * CH : (half + 1) * CH],
                        op0=max_op,
                        op1=mul_op,
                    )
                    if half == 1:
                        p0 = (c - 1) * CH
                        nc.vector.tensor_tensor(
                            out=hT[:, p0 : p0 + 2 * CH],
                            in0=cT[:, p0 : p0 + 2 * CH],
                            in1=sp3[:, 2, :],
                            op=mul_op,
                        )
                else:
                    # g-gate path MUST be on DVE: it reads pg from PSUM and
                    # GPSIMD/Pool instructions cannot access PSUM (BIR
                    # verifier rejects; the cost-model sim permits it).
                    g_inst = nc.vector.scalar_tensor_tensor(
                        out=t1p[:, half * CH : (half + 1) * CH],
                        in0=pg[:],
                        scalar=0.0,
                        in1=sp3[:, 0, half * CH : (half + 1) * CH],
                        op0=max_op,
                        op1=mul_op,
                    )
                    if last_h[0] is not None:
                        # ordering-only dep: h-writes (next-t critical path)
                        # beat this g-stt in the DVE ready heap.
                        g_inst.ins.add_dependency(
                            last_h[0], mybir.DependencyInfo.NO_SYNC_ONLY
                        )
                    if half == 1:
                        if t < T - 1:
                            # finish the pair EAGERLY: h(pair0) must be ready
                            # before the next timestep's U-matmuls for chunks
                            # 0/1, and the late ("one pair behind") order put
                            # it behind sigmoid(c2)/sigmoid(c3)-gated stts in
                            # the in-order DVE queue (~1.4us/t boundary stall).
                            finish_pair(c // 2, sp, t1p)
                        else:
                            # t == T-1: keep the late order so the dense psum
                            # tile can be the 5th gpool allocation (slot of
                            # pg(c=2)) -- there is no 9th psum bank.
                            if pending is not None:
                                finish_pair(*pending)
                                # dense psum as TWO tiles (5th/6th gpool
                                # allocations, slots of pg(c2)/pg(c3)) so
                                # the first softmax half doesn't wait on
                                # the second dense batch via a shared-tile
                                # dependency.
                                pgd0 = gpool.tile([H, CH], FP32, name="pg")
                                pw0 = pgd0[:, 0 : (BC // H) * NCLS // 2]
                                emit_dense(pw0, range(0, 8), 0)
                            pending = (c // 2, sp, t1p)
            if pending is not None:
                finish_pair(*pending)
            if t == T - 1:
                pgd1 = gpool.tile([H, CH], FP32, name="pg")
                pw1 = pgd1[:, 0 : (BC // H) * NCLS // 2]
                emit_dense(pw1, range(8, 16), 8)

        # softmax on the dense logits (emitted per-chunk inside t=T-1).
        # All 16 batch-blocks' logits land in ONE [128, 160] psum tile
        # (block j at cols 10j..10j+10).  Processed in TWO halves: blocks
        # 0..7 (whose dense matmuls finished with pair0 of T-1) run exp +
        # reduce + reciprocal + multiply + out-DMA while pair1's finish
        # chain and dense(8..16) are still in flight, halving the visible
        # tail.  DMAs go on different queues to overlap the transfers.
        NB = BC // H  # 16
        HB = NB // 2  # 8 blocks per half
        HW_ = HB * NCLS  # 80 cols per half
        # logits are O(1) (sigmoid-gated h, small Wd) -- skip max-subtract
        for halfb, pwh, q in ((0, pw0, nc.gpsimd), (1, pw1, None)):
            w0 = halfb * HW_
            ex = opool.tile([H, HW_], FP32)
            nc.scalar.activation(out=ex[:], in_=pwh[:], func=Exp)
            ex3 = ex[:].rearrange("p (g k) -> p g k", g=HB)
            sm = opool.tile([H, HB], FP32)
            nc.vector.tensor_reduce(
                out=sm[:], in_=ex3, axis=mybir.AxisListType.X, op=add_op
            )
            rc = opool.tile([H, HB], FP32)
            nc.vector.reciprocal(out=rc[:], in_=sm[:])
            # fp16 output halves the final DMA; probabilities are in [0,1]
            # so the added rounding is <= 2.5e-4 absolute (vs 8.9e-3 total).
            pr = opool.tile([H, HW_], FP16)
            nc.vector.tensor_tensor(
                out=pr[:].rearrange("p (g k) -> p g k", g=HB),
                in0=ex3,
                in1=rc[:].unsqueeze(2).broadcast_to([H, HB, NCLS]),
                op=mul_op,
            )
            # out DRAM is partition-major [128, NB*NCLS]: one contiguous
            # 160B descriptor per partition per half; the host undoes the
            # layout after gathering.
            (q or dma).dma_start(out=out[:, w0 : w0 + HW_], in_=pr[:])


def _build_nc(skip_bias):
    nc = bacc.Bacc(None, target_bir_lowering=False, debug=False)
    xT8 = nc.declare_dram_parameter("xT8", [E_FP8, FPAD, 2, BC], FP8, isOutput=False)
    xT16 = nc.declare_dram_parameter("xT16", [T - E_FP8, FPAD, BC], FP16, isOutput=False)
    Wdr0 = nc.declare_dram_parameter("Wdr0", [H, 2, H], FP8, isOutput=False)
    WdrR = nc.declare_dram_parameter("WdrR", [H, 2, G - H], FP8, isOutput=False)
    Wt16 = nc.declare_dram_parameter("Wt16", [H, G], FP16, isOutput=False)
    Ur = nc.declare_dram_parameter("Ur", [H, G], FP16, isOutput=False)
    Wd = nc.declare_dram_parameter("Wd", [H, NCLS], FP16, isOutput=False)
    bd = nc.declare_dram_parameter("bd", [1, NCLS], FP16, isOutput=False)
    ones1h = nc.declare_dram_parameter("ones1h", [1, H], FP16, isOutput=False)
    out = nc.declare_dram_parameter("out", [H, (BC // H) * NCLS], FP16, isOutput=True)

    with tile.TileContext(nc) as tc, ExitStack() as ctx:
        _build_kernel(
            ctx, tc, xT8, xT16, Wdr0, WdrR, Wt16, Ur, Wd, bd, ones1h, out, skip_bias
        )
    return nc


# psum/sigmoid gate order [i, f, o, g]; W/U columns are [i, f, g, o]
_GATE_PERM = np.concatenate(
    [np.arange(0, 2 * H), np.arange(3 * H, 4 * H), np.arange(2 * H, 3 * H)]
)


def _prepare_in_maps(x, W, U, b, Wd, bd):
    import ml_dtypes

    E4 = ml_dtypes.float8_e4m3
    bf16 = np.float16

    Wfull = np.vstack([W, b[None, :], np.zeros((H - FP, G), np.float32)])[:, _GATE_PERM]
    Wt16_host = np.ascontiguousarray(Wfull.astype(bf16))
    W8 = Wfull.astype(E4)
    Wlo = (Wfull / RSCALE).astype(E4)
    Wdr_host = np.ascontiguousarray(np.stack([W8, Wlo], axis=1))  # [H, 2, G]
    Ur_host = np.ascontiguousarray(U[:, _GATE_PERM].astype(bf16))
    Wd_host = np.ascontiguousarray(Wd.astype(bf16))
    bd_host = np.ascontiguousarray(bd.reshape(1, NCLS).astype(bf16))

    xs = x.reshape(NCORES, BC, T, F)
    in_maps = []
    for ci in range(NCORES):
        xc = xs[ci].transpose(1, 2, 0)  # [T, F, BC] fp32
        x8 = xc.astype(E4)
        xr8 = ((xc - x8.astype(np.float32)) * RSCALE).astype(E4)
        ones = np.ones((E_FP8, 1, BC), dtype=E4)
        zpad = np.zeros((E_FP8, FPAD - FP, BC), dtype=E4)
        zer1 = np.zeros((E_FP8, 1, BC), dtype=E4)
        p0 = np.concatenate([x8[:E_FP8], ones, zpad], axis=1)  # [E, FPAD, BC]
        p1 = np.concatenate([xr8[:E_FP8], zer1, zpad], axis=1)
        xT8c = np.ascontiguousarray(np.stack([p0, p1], axis=2))  # [E, FP, 2, BC]
        xT16c = np.ascontiguousarray(
            np.concatenate(
                [
                    xc[E_FP8:].astype(bf16),
                    np.ones((T - E_FP8, 1, BC), dtype=bf16),
                    np.zeros((T - E_FP8, FPAD - FP, BC), dtype=bf16),
                ],
                axis=1,
            )
        )  # [T-E, FPAD, BC]
        in_maps.append(
            {
                "xT8": xT8c,
                "xT16": xT16c,
                "Wdr0": np.ascontiguousarray(Wdr_host[:, :, 0:H]),
                "WdrR": np.ascontiguousarray(Wdr_host[:, :, H:G]),
                "Wt16": Wt16_host,
                "Ur": Ur_host,
                "Wd": Wd_host,
                "bd": bd_host,
                "ones1h": np.ones((1, H), dtype=bf16),
            }
        )
    return in_maps


def _run_timed(nc, in_maps, n_cores, reps):
    """Cached-executable min-of-N wall timing (NTFF unavailable under axon).

    Mirrors bass2jax.run_bass_via_pjrt's multi-core path but jits WITHOUT
    donation (our kernel writes every output element, so zero-init buffers
    are not needed) and keeps all operands device-resident across reps.
    """
    import time as _time

    import jax
    from jax.experimental.shard_map import shard_map
    from jax.sharding import Mesh, NamedSharding, PartitionSpec

    from concourse import bass2jax

    bass2jax.install_neuronx_cc_hook()
    partition_name = nc.partition_id_tensor.name if nc.partition_id_tensor else None

    in_names, out_names, out_avals, zero_outs = [], [], [], []
    for alloc in nc.m.functions[0].allocations:
        if not isinstance(alloc, mybir.MemoryLocationSet):
            continue
        name = alloc.memorylocations[0].name
        if alloc.kind == "ExternalInput":
            if name != partition_name:
                in_names.append(name)
        elif alloc.kind == "ExternalOutput":
            out_names.append(name)
            shape = tuple(alloc.tensor_shape)
            dtype = mybir.dt.np(alloc.dtype)
            out_avals.append(jax.core.ShapedArray(shape, dtype))
            zero_outs.append(np.zeros(shape, dtype))
    n_params = len(in_names)
    in_names = in_names + out_names
    if partition_name is not None:
        in_names.append(partition_name)

    def _body(*args):
        operands = list(args)
        if partition_name is not None:
            operands.append(bass2jax.partition_id_tensor())
        return tuple(
            bass2jax._bass_exec_p.bind(
                *operands,
                out_avals=tuple(out_avals),
                in_names=tuple(in_names),
                out_names=tuple(out_names),
                lowering_input_output_aliases=(),
                sim_require_finite=True,
                sim_require_nnan=True,
                nc=nc,
            )
        )

    devices = jax.devices()[:n_cores]
    mesh = Mesh(np.asarray(devices), ("core",))
    nsh = NamedSharding(mesh, PartitionSpec("core"))
    in_specs = (PartitionSpec("core"),) * (n_params + len(out_names))
    out_specs = (PartitionSpec("core"),) * len(out_names)
    sharded = jax.jit(
        shard_map(
            _body, mesh=mesh, in_specs=in_specs, out_specs=out_specs, check_rep=False
        ),
        keep_unused=True,
    )
    per_core = [[np.asarray(m[name]) for name in in_names[:n_params]] for m in in_maps]
    concat_in = [
        np.concatenate([per_core[c][i] for c in range(n_cores)], axis=0)
        for i in range(n_params)
    ]
    concat_zeros = [
        np.zeros((n_cores * z.shape[0], *z.shape[1:]), z.dtype) for z in zero_outs
    ]
    args_dev = [jax.device_put(a, nsh) for a in concat_in + concat_zeros]
    out = jax.block_until_ready(sharded(*args_dev))  # compile + warmup
    times = []
    for _ in range(reps):
        t0 = _time.perf_counter_ns()
        o = jax.block_until_ready(sharded(*args_dev))
        times.append(_time.perf_counter_ns() - t0)
    results = [
        {
            name: np.asarray(out[i]).reshape(n_cores, *out_avals[i].shape)[c]
            for i, name in enumerate(out_names)
        }
        for c in range(n_cores)
    ]
    return results, min(times), sum(times) / len(times)


def kernel(x, W, U, b, Wd, bd):
    global LAST_RESULT
    x = np.ascontiguousarray(np.asarray(x, dtype=np.float32))
    W = np.asarray(W, dtype=np.float32)
    U = np.asarray(U, dtype=np.float32)
    b = np.asarray(b, dtype=np.float32)
    Wd = np.ascontiguousarray(np.asarray(Wd, dtype=np.float32))
    bd = np.asarray(bd, dtype=np.float32)

    in_maps = _prepare_in_maps(x, W, U, b, Wd, bd)

    nc = _build_nc(skip_bias=not np.any(bd))
    nc.finalize()
    if TIME_REPS > 0:
        from concourse.bass_utils import BassKernelResults

        results, min_ns, mean_ns = _run_timed(nc, in_maps, NCORES, TIME_REPS)
        res = BassKernelResults(
            results=results,
            instructions_and_trace=None,
            profile_json=None,
            exec_time_ns=int(min_ns),
            mean_exec_time_ns=mean_ns,
        )
    else:
        res = run_bass_kernel_spmd(nc, in_maps, list(range(NCORES)), trace=TRACE)
    LAST_RESULT = res
    NB = BC // H
    outs = []
    for i in range(NCORES):
        oc = np.asarray(res.results[i]["out"], dtype=np.float32).reshape(H, NB, NCLS)
        outs.append(oc.transpose(1, 0, 2).reshape(BC, NCLS))  # batch b = g*128 + p
    out = np.concatenate(outs, axis=0)
    return np.ascontiguousarray(out.astype(np.float32))


# revision 49
# speedup vs baseline: 1.0100x; 1.0100x over previous
"""Bass/Trainium2 kernel for the LSTM problem (nn_RNN_27685359190558).

Math (per reference):
  xW = x @ W + b                      [B, T, 4H]
  scan over T=28: z = xW_t + h @ U; i,f,g,o = split(z) (Keras order)
      i,f,o = sigmoid; g = relu
      c' = f*c + i*g;  h' = o * relu(c')
  out = softmax(h_final @ Wd + bd)    [B, 10]

Strategy: pure data parallelism over 8 cores (2048 batch each).
On-chip layout is fully transposed: states hT/cT are [H=128 partitions,
batch free].  Per (timestep, 512-batch chunk) and per gate q:
psum[q] = Ur[:,q].T @ hT (fp16) + W-side x-projection.

fp8 DoubleRow x-projection (CoreSim 229us -> 177us total vs the fp16
baseline; HW-verified rel err 8.8e-3 against the 2e-2 gate):
- For timesteps 0..E_FP8-1 the x-projection is ONE fp8e4 DoubleRow
  matmul per gate: plane0 = e4m3(W).T @ e4m3(x), plane1 = e4m3(W/16).T
  @ e4m3(16*(x - e4m3(x))).  The residual plane restores x to ~fp16
  accuracy; only W's ~3% quantization error remains.  DoubleRow = 0.5
  cycles/row (vs 1.0 fp16) at out free-size 512 >= the FD>=256 the HW
  needs for its ~1.4x win; it sums the two (lhsT plane_i, rhs plane_i)
  products, so rhs interleaves [x8 | xr8] planes per tile.
- Early-step quantization errors decay through the forget-gate products;
  the LAST step's error hits h_final undamped.  E_FP8=27 (all but t=27)
  measures 8.8e-3; E=24 gives 3.3e-3; E=28 gives 1.5e-2.  t=27 stays
  fully fp16.

Engine assignment (all BIR-verifier legal; GPSIMD may NOT touch PSUM):
- ACT: one fused sigmoid per chunk over [i,f,o] psum [128,1536] -- the
  binding engine (94% busy in sim; 1465ns per sigmoid, 164us total).
- DVE: g-gate stt t1 = relu(pg_psum)*sig_i (PSUM-capable engine
  required), plus pair-wide c += t1 and h = c*o tensor_tensors.
  NOTE: scalar_tensor_tensor gets NO DVE fast mode; tensor_tensor gets
  2x_1p only (594ns/1024 fp16); 4x_2p exists only for tensor_scalar.
- Pool/GPSIMD: c = f*c (SBUF-only operands, 853ns/1024) in parallel
  with the g-stt -- shortens the serial post-sigmoid segment of the
  recurrence lane; x-tile zeroing via DVE AND-0 (gpsimd memset of a
  4KB/partition tile costs 3.4us and serialized the first x DMA).
- no-sync (ordering-only) deps pin the DVE ready-heap: h-writes (the
  next timestep's U-matmul input) must not lose to later g-stts.

Schedule notes:
- Per chunk, ALL x-side matmuls go first (depend only on prefetched x),
  U-matmuls close the psum groups, U(g) last -- sigmoid reads only pt.
- x prefetch depth 3 on SP; 4 rotating [128, 2, 2048B] buffers whose
  bytes serve fp8 steps directly and the fp16 step via a bitcast view.
- PE warmup matmuls burn the DMA preamble ramping the HAM p-state.
- Dense psum is TWO half tiles so softmax half 0 (+ its fp16 out-DMA on
  the Pool queue) overlaps pair1's finish chain and dense(8..16).
- Since c0=0 and c' = f*c + i*relu(g) with f,i>0, c stays >= 0, so
  relu(c)=c and h' = o*c is a plain multiply.
- Bias b is folded in via a ones-row appended to x (host side); x rows
  are padded 29->32 for DMA partition alignment, rows 32..127 zeroed
  once (zero W rows make them inert).

Kept from earlier HW-measured findings: fp16 (not fp32r) everywhere
on the non-fp8 path; K padded to 128 (K<128 streams at half rate);
1024-wide DVE ops (512-wide collapses ~3x under concurrent traffic).
"""

import sys

sys.path.insert(0, "/opt/trn_rl_repo")

import numpy as np
from contextlib import ExitStack

import concourse.bass as bass
import concourse.bacc as bacc
import concourse.tile as tile
from concourse import mybir
from concourse.bass_utils import run_bass_kernel_spmd

B, T, F, H = 16384, 28, 28, 128
G = 4 * H  # 512
NCLS = 10
NCORES = 8
BC = B // NCORES  # 2048 batch per core
CH = 512  # batch chunk per matmul (one psum bank)
NCH = BC // CH  # 4
FP = F + 1  # 29: features + ones row (bias)
FPAD = 32  # DMA row count: FP padded to the 32-partition alignment
E_FP8 = 27  # timesteps 0..E_FP8-1 use the fp8 DoubleRow x-projection
RSCALE = 16.0  # residual-plane upscale

FP32 = mybir.dt.float32
FP16 = mybir.dt.float16
FP8 = mybir.dt.float8e4  # e4m3

TRACE = False
TIME_REPS = 0  # >0: run cached-executable wall-clock timing after correctness run
LAST_RESULT = None

DR = mybir.MatmulPerfMode.DoubleRow


def _build_kernel(ctx, tc, xT8, xT16, Wdr0, WdrR, Wt16, Ur, Wd, bd, ones1h, out, skip_bias):
    nc = tc.nc
    Sig = mybir.ActivationFunctionType.Sigmoid
    Exp = mybir.ActivationFunctionType.Exp
    mul_op = mybir.AluOpType.mult
    add_op = mybir.AluOpType.add
    max_op = mybir.AluOpType.max

    weights = ctx.enter_context(tc.tile_pool(name="weights", bufs=1))
    state = ctx.enter_context(tc.tile_pool(name="state", bufs=1))
    xpool = ctx.enter_context(tc.tile_pool(name="xpool", bufs=1))
    spool = ctx.enter_context(tc.tile_pool(name="spool", bufs=8))
    tpool = ctx.enter_context(tc.tile_pool(name="tpool", bufs=8))
    opool = ctx.enter_context(tc.tile_pool(name="opool", bufs=2))

    dma = nc.default_dma_engine

    wdr_sb = weights.tile([H, 2, G], FP8)  # plane0 = W8, plane1 = Wlo (x-residual)
    wt_sb = weights.tile([H, G], FP16)  # fp16 W for the tail timesteps
    ur_sb = weights.tile([H, G], FP16)
    wd_sb = weights.tile([H, NCLS], FP16)
    bd_sb = weights.tile([1, NCLS], FP16)
    ones_sb = weights.tile([1, H], FP16)

    # DMA order = queue order.  The ACT queue head carries the activation
    # table loads (~1.3us each), so the two tensors the FIRST matmul needs
    # (DR weights gate i + x0 chunk 0) go at the heads of the SP and Pool
    # queues instead; everything else follows on SP.
    dma.dma_start(out=wdr_sb[:, :, 0:H], in_=Wdr0[:])
    # x tiles: [H partitions, 2 planes, BC] fp8.  Rows FP..127 stay zero
    # (memset once); DMA rewrites rows 0..FP-1 each use.  The same bytes
    # serve the fp16 tail steps via a bitcast view [H, BC] fp16.
    xbufs = [xpool.tile([H, 2, BC], FP8, name=f"xtbuf{j}") for j in range(4)]
    xv16 = [
        xb[:].rearrange("p a b -> p (a b)").bitcast(FP16) for xb in xbufs
    ]  # [H, BC] fp16 views
    warm_src = weights.tile([H, H], FP16)
    nc.gpsimd.memset(warm_src[:], 0.0)
    # Zero the x tiles on DVE instead of gpsimd: the 3.4us-per-buffer Pool
    # memsets serialized the first x DMA to t=3.7us.  DVE (in0 AND 0.0)
    # writes exact zeros even from uninitialized (possibly NaN) bits in
    # 593ns per buffer -- only the pad rows FP..127 actually need it, but a
    # partition slice starting above 0 is capped at 32 partitions, so zero
    # the full tile; the brief x0-DMA WAR wait is negligible.
    for j in range(4):
        full = xbufs[j][:].rearrange("p a b -> p (a b)").bitcast(FP16)
        nc.vector.tensor_scalar(
            out=full,
            in0=full,
            scalar1=0.0,
            scalar2=None,
            op0=mybir.AluOpType.logical_and,
        )
    xt0 = xbufs[0]
    nc.gpsimd.dma_start(out=xt0[0:FPAD, :, 0:CH], in_=xT8[0][:, :, 0:CH])
    for c in range(1, NCH):
        dma.dma_start(
            out=xt0[0:FPAD, :, c * CH : (c + 1) * CH],
            in_=xT8[0][:, :, c * CH : (c + 1) * CH],
        )
    dma.dma_start(out=wdr_sb[:, :, H:G], in_=WdrR[:])
    # prefetch t=1/t=2's x right behind t=0's; t>=3 prefetches issue inside
    # the t-loop three timesteps ahead on the Pool queue.
    dma.dma_start(out=xbufs[1][0:FPAD, :, :], in_=xT8[1])
    dma.dma_start(out=xbufs[2][0:FPAD, :, :], in_=xT8[2])
    dma.dma_start(out=ur_sb[:], in_=Ur[:])
    dma.dma_start(out=wt_sb[:], in_=Wt16[:])
    dma.dma_start(out=wd_sb[:], in_=Wd[:])
    dma.dma_start(out=bd_sb[:], in_=bd[:])
    dma.dma_start(out=ones_sb[:], in_=ones1h[:])

    hT = state.tile([H, BC], FP16)
    cT = state.tile([H, BC], FP16)

    last_h = [None]  # name of the most recent h-producing tt (see finish_pair)

    def finish_pair(p, sp, t1p):
        # Pair-wide (1024-elem) c = f*c, c += i*relu(g), h = c*o -- on DVE
        # tensor_tensor (2x_1p mode; scalar_tensor_tensor gets NO fast mode,
        # so fusing ops into stt is a net loss).  The h write is on the next
        # timestep's critical path (U-matmuls read it), so later g-stts get
        # a no-sync dep on it to keep the ready-heap from reordering.
        # c=f*c runs on the otherwise-idle Pool engine (SBUF-only operands
        # -- legal on GPSIMD; measured 853ns/1024) IN PARALLEL with the
        # g-stt on DVE, shortening the serial post-sigmoid segment on the
        # recurrence lane and relieving the DVE queue.
        p0, p1 = p * 2 * CH, (p + 1) * 2 * CH
        sp3 = sp[:].rearrange("h (g w) -> h g w", g=3)
        eng = nc.gpsimd
        i1 = eng.tensor_tensor(
            out=cT[:, p0:p1], in0=sp3[:, 1, :], in1=cT[:, p0:p1], op=mul_op
        )
        i2 = nc.vector.tensor_tensor(
            out=cT[:, p0:p1], in0=cT[:, p0:p1], in1=t1p[:], op=add_op
        )
        i3 = nc.vector.tensor_tensor(
            out=hT[:, p0:p1], in0=cT[:, p0:p1], in1=sp3[:, 2, :], op=mul_op
        )
        last_h[0] = i3.ins.name

    def finish_single(c, sp, t1p):
        # tail-only per-chunk finish (shorter drain at t=T-1)
        c0, c1 = c * CH, (c + 1) * CH
        half = c % 2
        sp3 = sp[:].rearrange("h (g w) -> h g w", g=3)
        nc.vector.tensor_tensor(
            out=cT[:, c0:c1],
            in0=cT[:, c0:c1],
            in1=t1p[:, half * CH : (half + 1) * CH],
            op=add_op,
        )
        nc.vector.tensor_tensor(
            out=hT[:, c0:c1],
            in0=cT[:, c0:c1],
            in1=sp3[:, 2, half * CH : (half + 1) * CH],
            op=mul_op,
        )

    def emit_dense(pw, blocks, j0):
        for j in blocks:
            d0 = (j - j0) * NCLS
            nc.tensor.matmul(
                pw[:, d0 : d0 + NCLS],
                hT[:, j * H : (j + 1) * H],
                wd_sb[:],
                start=True,
                stop=skip_bias,
            )
            if not skip_bias:
                # + bd via a rank-1 ones @ bd matmul (keeps bias off the DVE)
                nc.tensor.matmul(
                    pw[:, d0 : d0 + NCLS], ones_sb[:], bd_sb[:], start=False, stop=True
                )

    with (
        tc.tile_pool(name="ppool", bufs=2, space="PSUM") as ppool,
        tc.tile_pool(name="gpool", bufs=2, space="PSUM") as gpool,
    ):
        for t in range(T):
            xt = xbufs[t % 4]
            if t + 3 < T:
                # x prefetch at depth 3 (4 rotating bufs): a full timestep of
                # slack so the SP-queue transfer is never on the boundary
                # critical path (depth 2 landed just-in-time).
                j = (t + 3) % 4
                if t + 3 < E_FP8:
                    dma.dma_start(out=xbufs[j][0:FPAD, :, :], in_=xT8[t + 3])
                else:
                    dma.dma_start(out=xv16[j][0:FPAD, :], in_=xT16[t + 3 - E_FP8])
            pending = None
            for c in range(NCH):
                c0, c1 = c * CH, (c + 1) * CH
                half = c % 2
                if half == 0:
                    # s for a chunk PAIR, gate-major: [i0 i1 | f0 f1 | o0 o1]
                    sp = spool.tile([H, 3 * 2 * CH], FP16)
                    t1p = tpool.tile([H, 2 * CH], FP16)
                sp3 = sp[:].rearrange("h (g w) -> h g w", g=3)
                pt = ppool.tile([H, 3 * CH], FP32)
                pg = gpool.tile([H, CH], FP32)

                if t == 0 and c == 0:
                    # PE warmup during the DMA preamble (see warm_src note):
                    # dummy matmuls into the first pt tile; the real DR
                    # matmuls below open with start=True, resetting psum.
                    for _ in range(12):
                        nc.tensor.matmul(
                            pt[:, 0:H],
                            warm_src[:],
                            warm_src[:],
                            start=True,
                            stop=True,
                        )

                # All x-side matmuls FIRST: they depend only on the
                # (prefetched) x DMA, so at the timestep boundary PE streams
                # the x projection while the previous timestep's h is still
                # in flight on DVE.  U-matmuls (h-gated) close the groups;
                # U(g) goes LAST because sigmoid only reads pt (gates i,f,o)
                # -- the g group (pg, read by the Pool stt) can close after
                # sigmoid already started.
                def xdst(q):
                    return pt[:, q * CH : (q + 1) * CH] if q < 3 else pg[:]

                for q in range(4):
                    if t < E_FP8:
                        nc.tensor.matmul(
                            xdst(q),
                            wdr_sb[:, :, q * H : (q + 1) * H],
                            xt[:, :, c0:c1],
                            start=True,
                            stop=(t == 0),
                            perf_mode=DR,
                        )
                    else:
                        nc.tensor.matmul(
                            xdst(q),
                            wt_sb[:, q * H : (q + 1) * H],
                            xv16[t % 4][:, c0:c1],
                            start=True,
                            stop=(t == 0),
                        )
                if t > 0:
                    for q in range(4):
                        nc.tensor.matmul(
                            xdst(q),
                            ur_sb[:, q * H : (q + 1) * H],
                            hT[:, c0:c1],
                            start=False,
                            stop=True,
                        )
                sig_inst = nc.scalar.activation(
                    out=sp3[:, :, half * CH : (half + 1) * CH],
                    in_=pt[:].rearrange("h (g w) -> h g w", g=3),
                    func=Sig,
                )
                if t == 0:
                    # c0 = 0  =>  c' = i * relu(g) = relu(i*g);  h = o*c
                    nc.vector.scalar_tensor_tensor(
                        out=cT[:, c0:c1],
                        in0=pg[:],
                        scalar=0.0,
                        in1=sp3[:, 0, half * CH : (half + 1) * CH],
                        op0=max_op,
                        op1=mul_op,
                    )
                    if half == 1:
                        p0 = (c - 1) * CH
                        nc.vector.tensor_tensor(
                            out=hT[:, p0 : p0 + 2 * CH],
                            in0=cT[:, p0 : p0 + 2 * CH],
                            in1=sp3[:, 2, :],
                            op=mul_op,
                        )
                else:
                    # g-gate path MUST be on DVE: it reads pg from PSUM and
                    # GPSIMD/Pool instructions cannot access PSUM (BIR
                    # verifier rejects; the cost-model sim permits it).
                    g_inst = nc.vector.scalar_tensor_tensor(
                        out=t1p[:, half * CH : (half + 1) * CH],
                        in0=pg[:],
                        scalar=0.0,
                        in1=sp3[:, 0, half * CH : (half + 1) * CH],
                        op0=max_op,
                        op1=mul_op,
                    )
                    if last_h[0] is not None:
                        # ordering-only dep: h-writes (next-t critical path)
                        # beat this g-stt in the DVE ready heap.
                        g_inst.ins.add_dependency(
                            last_h[0], mybir.DependencyInfo.NO_SYNC_ONLY
                        )
                    if half == 1:
                        if t < T - 1:
                            # finish the pair EAGERLY: h(pair0) must be ready
                            # before the next timestep's U-matmuls for chunks
                            # 0/1, and the late ("one pair behind") order put
                            # it behind sigmoid(c2)/sigmoid(c3)-gated stts in
                            # the in-order DVE queue (~1.4us/t boundary stall).
                            finish_pair(c // 2, sp, t1p)
                        else:
                            # t == T-1: keep the late order so the dense psum
                            # tile can be the 5th gpool allocation (slot of
                            # pg(c=2)) -- there is no 9th psum bank.
                            if pending is not None:
                                finish_pair(*pending)
                                # dense psum as TWO tiles (5th/6th gpool
                                # allocations, slots of pg(c2)/pg(c3)) so
                                # the first softmax half doesn't wait on
                                # the second dense batch via a shared-tile
                                # dependency.
                                pgd0 = gpool.tile([H, CH], FP32, name="pg")
                                pw0 = pgd0[:, 0 : (BC // H) * NCLS // 2]
                                emit_dense(pw0, range(0, 8), 0)
                            pending = (c // 2, sp, t1p)
            if pending is not None:
                finish_pair(*pending)
            if t == T - 1:
                pgd1 = gpool.tile([H, CH], FP32, name="pg")
                pw1 = pgd1[:, 0 : (BC // H) * NCLS // 2]
                emit_dense(pw1, range(8, 16), 8)

        # softmax on the dense logits (emitted per-chunk inside t=T-1).
        # All 16 batch-blocks' logits land in ONE [128, 160] psum tile
        # (block j at cols 10j..10j+10).  Processed in TWO halves: blocks
        # 0..7 (whose dense matmuls finished with pair0 of T-1) run exp +
        # reduce + reciprocal + multiply + out-DMA while pair1's finish
        # chain and dense(8..16) are still in flight, halving the visible
        # tail.  DMAs go on different queues to overlap the transfers.
        NB = BC // H  # 16
        HB = NB // 2  # 8 blocks per half
        HW_ = HB * NCLS  # 80 cols per half
        # logits are O(1) (sigmoid-gated h, small Wd) -- skip max-subtract
        for halfb, pwh, q in ((0, pw0, nc.gpsimd), (1, pw1, None)):
            w0 = halfb * HW_
            ex = opool.tile([H, HW_], FP32)
            nc.scalar.activation(out=ex[:], in_=pwh[:], func=Exp)
            ex3 = ex[:].rearrange("p (g k) -> p g k", g=HB)
            sm = opool.tile([H, HB], FP32)
            red = nc.vector.tensor_reduce(
                out=sm[:], in_=ex3, axis=mybir.AxisListType.X, op=add_op
            )
            if halfb == 0 and last_h[0] is not None:
                # ordering-only: don't let half0's reduce slip between the
                # T-1 pair1 c/h tts on the DVE queue (h gates dense(8..16)).
                red.ins.add_dependency(
                    last_h[0], mybir.DependencyInfo.NO_SYNC_ONLY
                )
            rc = opool.tile([H, HB], FP32)
            nc.vector.reciprocal(out=rc[:], in_=sm[:])
            # fp16 output halves the final DMA; probabilities are in [0,1]
            # so the added rounding is <= 2.5e-4 absolute (vs 8.9e-3 total).
            pr = opool.tile([H, HW_], FP16)
            nc.vector.tensor_tensor(
                out=pr[:].rearrange("p (g k) -> p g k", g=HB),
                in0=ex3,
                in1=rc[:].unsqueeze(2).broadcast_to([H, HB, NCLS]),
                op=mul_op,
            )
            # out DRAM is partition-major [128, NB*NCLS]: one contiguous
            # 160B descriptor per partition per half; the host undoes the
            # layout after gathering.
            (q or dma).dma_start(out=out[:, w0 : w0 + HW_], in_=pr[:])


def _build_nc(skip_bias):
    nc = bacc.Bacc(None, target_bir_lowering=False, debug=False)
    xT8 = nc.declare_dram_parameter("xT8", [E_FP8, FPAD, 2, BC], FP8, isOutput=False)
    xT16 = nc.declare_dram_parameter("xT16", [T - E_FP8, FPAD, BC], FP16, isOutput=False)
    Wdr0 = nc.declare_dram_parameter("Wdr0", [H, 2, H], FP8, isOutput=False)
    WdrR = nc.declare_dram_parameter("WdrR", [H, 2, G - H], FP8, isOutput=False)
    Wt16 = nc.declare_dram_parameter("Wt16", [H, G], FP16, isOutput=False)
    Ur = nc.declare_dram_parameter("Ur", [H, G], FP16, isOutput=False)
    Wd = nc.declare_dram_parameter("Wd", [H, NCLS], FP16, isOutput=False)
    bd = nc.declare_dram_parameter("bd", [1, NCLS], FP16, isOutput=False)
    ones1h = nc.declare_dram_parameter("ones1h", [1, H], FP16, isOutput=False)
    out = nc.declare_dram_parameter("out", [H, (BC // H) * NCLS], FP16, isOutput=True)

    with tile.TileContext(nc) as tc, ExitStack() as ctx:
        _build_kernel(
            ctx, tc, xT8, xT16, Wdr0, WdrR, Wt16, Ur, Wd, bd, ones1h, out, skip_bias
        )
    return nc


# psum/sigmoid gate order [i, f, o, g]; W/U columns are [i, f, g, o]
_GATE_PERM = np.concatenate(
    [np.arange(0, 2 * H), np.arange(3 * H, 4 * H), np.arange(2 * H, 3 * H)]
)


def _prepare_in_maps(x, W, U, b, Wd, bd):
    import ml_dtypes

    E4 = ml_dtypes.float8_e4m3
    bf16 = np.float16

    Wfull = np.vstack([W, b[None, :], np.zeros((H - FP, G), np.float32)])[:, _GATE_PERM]
    Wt16_host = np.ascontiguousarray(Wfull.astype(bf16))
    W8 = Wfull.astype(E4)
    Wlo = (Wfull / RSCALE).astype(E4)
    Wdr_host = np.ascontiguousarray(np.stack([W8, Wlo], axis=1))  # [H, 2, G]
    Ur_host = np.ascontiguousarray(U[:, _GATE_PERM].astype(bf16))
    Wd_host = np.ascontiguousarray(Wd.astype(bf16))
    bd_host = np.ascontiguousarray(bd.reshape(1, NCLS).astype(bf16))

    xs = x.reshape(NCORES, BC, T, F)
    in_maps = []
    for ci in range(NCORES):
        xc = xs[ci].transpose(1, 2, 0)  # [T, F, BC] fp32
        x8 = xc.astype(E4)
        xr8 = ((xc - x8.astype(np.float32)) * RSCALE).astype(E4)
        ones = np.ones((E_FP8, 1, BC), dtype=E4)
        zpad = np.zeros((E_FP8, FPAD - FP, BC), dtype=E4)
        zer1 = np.zeros((E_FP8, 1, BC), dtype=E4)
        p0 = np.concatenate([x8[:E_FP8], ones, zpad], axis=1)  # [E, FPAD, BC]
        p1 = np.concatenate([xr8[:E_FP8], zer1, zpad], axis=1)
        xT8c = np.ascontiguousarray(np.stack([p0, p1], axis=2))  # [E, FP, 2, BC]
        xT16c = np.ascontiguousarray(
            np.concatenate(
                [
                    xc[E_FP8:].astype(bf16),
                    np.ones((T - E_FP8, 1, BC), dtype=bf16),
                    np.zeros((T - E_FP8, FPAD - FP, BC), dtype=bf16),
                ],
                axis=1,
            )
        )  # [T-E, FPAD, BC]
        in_maps.append(
            {
                "xT8": xT8c,
                "xT16": xT16c,
                "Wdr0": np.ascontiguousarray(Wdr_host[:, :, 0:H]),
                "WdrR": np.ascontiguousarray(Wdr_host[:, :, H:G]),
                "Wt16": Wt16_host,
                "Ur": Ur_host,
                "Wd": Wd_host,
                "bd": bd_host,
                "ones1h": np.ones((1, H), dtype=bf16),
            }
        )
    return in_maps


def _run_timed(nc, in_maps, n_cores, reps):
    """Cached-executable min-of-N wall timing (NTFF unavailable under axon).

    Mirrors bass2jax.run_bass_via_pjrt's multi-core path but jits WITHOUT
    donation (our kernel writes every output element, so zero-init buffers
    are not needed) and keeps all operands device-resident across reps.
    """
    import time as _time

    import jax
    from jax.experimental.shard_map import shard_map
    from jax.sharding import Mesh, NamedSharding, PartitionSpec

    from concourse import bass2jax

    bass2jax.install_neuronx_cc_hook()
    partition_name = nc.partition_id_tensor.name if nc.partition_id_tensor else None

    in_names, out_names, out_avals, zero_outs = [], [], [], []
    for alloc in nc.m.functions[0].allocations:
        if not isinstance(alloc, mybir.MemoryLocationSet):
            continue
        name = alloc.memorylocations[0].name
        if alloc.kind == "ExternalInput":
            if name != partition_name:
                in_names.append(name)
        elif alloc.kind == "ExternalOutput":
            out_names.append(name)
            shape = tuple(alloc.tensor_shape)
            dtype = mybir.dt.np(alloc.dtype)
            out_avals.append(jax.core.ShapedArray(shape, dtype))
            zero_outs.append(np.zeros(shape, dtype))
    n_params = len(in_names)
    in_names = in_names + out_names
    if partition_name is not None:
        in_names.append(partition_name)

    def _body(*args):
        operands = list(args)
        if partition_name is not None:
            operands.append(bass2jax.partition_id_tensor())
        return tuple(
            bass2jax._bass_exec_p.bind(
                *operands,
                out_avals=tuple(out_avals),
                in_names=tuple(in_names),
                out_names=tuple(out_names),
                lowering_input_output_aliases=(),
                sim_require_finite=True,
                sim_require_nnan=True,
                nc=nc,
            )
        )

    devices = jax.devices()[:n_cores]
    mesh = Mesh(np.asarray(devices), ("core",))
    nsh = NamedSharding(mesh, PartitionSpec("core"))
    in_specs = (PartitionSpec("core"),) * (n_params + len(out_names))
    out_specs = (PartitionSpec("core"),) * len(out_names)
    sharded = jax.jit(
        shard_map(
            _body, mesh=mesh, in_specs=in_specs, out_specs=out_specs, check_rep=False
        ),
        keep_unused=True,
    )
    per_core = [[np.asarray(m[name]) for name in in_names[:n_params]] for m in in_maps]
    concat_in = [
        np.concatenate([per_core[c][i] for c in range(n_cores)], axis=0)
        for i in range(n_params)
    ]
    concat_zeros = [
        np.zeros((n_cores * z.shape[0], *z.shape[1:]), z.dtype) for z in zero_outs
    ]
    args_dev = [jax.device_put(a, nsh) for a in concat_in + concat_zeros]
    out = jax.block_until_ready(sharded(*args_dev))  # compile + warmup
    times = []
    for _ in range(reps):
        t0 = _time.perf_counter_ns()
        o = jax.block_until_ready(sharded(*args_dev))
        times.append(_time.perf_counter_ns() - t0)
    results = [
        {
            name: np.asarray(out[i]).reshape(n_cores, *out_avals[i].shape)[c]
            for i, name in enumerate(out_names)
        }
        for c in range(n_cores)
    ]
    return results, min(times), sum(times) / len(times)


def kernel(x, W, U, b, Wd, bd):
    global LAST_RESULT
    x = np.ascontiguousarray(np.asarray(x, dtype=np.float32))
    W = np.asarray(W, dtype=np.float32)
    U = np.asarray(U, dtype=np.float32)
    b = np.asarray(b, dtype=np.float32)
    Wd = np.ascontiguousarray(np.asarray(Wd, dtype=np.float32))
    bd = np.asarray(bd, dtype=np.float32)

    in_maps = _prepare_in_maps(x, W, U, b, Wd, bd)

    nc = _build_nc(skip_bias=not np.any(bd))
    nc.finalize()
    if TIME_REPS > 0:
        from concourse.bass_utils import BassKernelResults

        results, min_ns, mean_ns = _run_timed(nc, in_maps, NCORES, TIME_REPS)
        res = BassKernelResults(
            results=results,
            instructions_and_trace=None,
            profile_json=None,
            exec_time_ns=int(min_ns),
            mean_exec_time_ns=mean_ns,
        )
    else:
        res = run_bass_kernel_spmd(nc, in_maps, list(range(NCORES)), trace=TRACE)
    LAST_RESULT = res
    NB = BC // H
    outs = []
    for i in range(NCORES):
        oc = np.asarray(res.results[i]["out"], dtype=np.float32).reshape(H, NB, NCLS)
        outs.append(oc.transpose(1, 0, 2).reshape(BC, NCLS))  # batch b = g*128 + p
    out = np.concatenate(outs, axis=0)
    return np.ascontiguousarray(out.astype(np.float32))
